# revision 23
# baseline (speedup 1.0000x reference)
"""ExpanderSAGE GNN kernel for 8x Trainium2 NeuronCores (Bass/Tile), v3.

Strategy (graph/data parallel, dst-sharded):
  - 50000 nodes sharded 6250/core (8 cores). Each core owns the edges whose
    dst lands in its shard, sorted into (block of 4 dst-windows, src-half,
    window, 128-edge chunk) order.
  - Layer 1 neighbor rows (x[src]) are pre-gathered on the HOST into the
    exact SBUF chunk layout (pure indexing, like the idx/dslot arrays), so
    the device streams them with big sequential DMAs - no descriptor
    generation.  Layers 2/3 use gpsimd.dma_gather, MAXC chunks per call
    (the SWDGE ring caps this near 1024 descriptors - MAXC>7 hangs).
  - Segment-sum via one-hot matmul, FEATURE-MAJOR: the gathered chunk is the
    stationary operand, the (edge -> dst slot) one-hot streams, so the psum
    comes out [feature, dst] and feeds the dense matmuls directly (no PE
    transposes / copies).  1/deg scaling happens during psum evacuation via
    a replicated inv-degree row (invdT).
  - v3: the gathered-neighbor path runs in f8e4m3 (xe stream, h1 gather
    table, one-hot) - halves the L1 stream, L2 gather wire, and h1
    AllGather bytes; measured rel err 1.9e-3 (gate 2e-2).  Both segsum
    operands must share the f8 dtype (mixed f8xf16 matmul measured slow);
    MatmulPerfMode.DoubleRow also measured slower - both stay off.  Dense
    math stays f16 with f32 psum accumulate; the L3 p-table stays f16.
  - BatchNorm stats from the f32 psum; AllReduce(8) of [128,4] partials.
    Layer 3 pre-projects p = h2 @ w3l (47 -> 64, stored f16 padded to 128
    cols = 256B rows) and AllGathers it; log_softmax runs node-major.
  - The timing-replica loop is software-pipelined (SOFTPIPE=1, 2-phase):
    rep k+1's whole layer-1 (incl. its stats-AllReduce and h1-AllGather
    issue) is emitted between rep k's p-AllGather and rep k's L3
    consumers, so collective latency overlaps compute in steady state.
    Measured in-process at R=12: seq 1442us, 2-phase 1164us, 4-way split
    (SOFTPIPE=2, ARs issued before the big AG) 1524us - the 4-way split
    shrinks the work window that hides the p-AllGather; keep 2-phase.
"""

import os
import sys

import numpy as np

for _p in ("/opt/trn_rl_repo", os.path.expanduser("~/.axon_site/_ro/trn_rl_repo")):
    if os.path.isdir(_p) and _p not in sys.path:
        sys.path.insert(0, _p)

import concourse.bacc as bacc
import concourse.mybir as mybir
import concourse.tile as tile
from concourse.bass_utils import run_bass_kernel_spmd
from concourse.masks import make_identity

F32 = mybir.dt.float32
F16 = mybir.dt.float16
F8 = mybir.dt.float8e4
I16 = mybir.dt.int16
I32 = mybir.dt.int32
AL = mybir.AluOpType
AF = mybir.ActivationFunctionType

EPS = 1e-5
NCORES = 8
P = 128
BPB = 4            # dst windows per block
BLK = BPB * P      # node block for dense matmuls
MAXC = int(os.environ.get("MAXC", "6"))  # chunks per dma_gather call (1024-desc SWDGE ring)
DSCRATCH = int(os.environ.get("DSCRATCH", "16384"))  # SWDGE scratch bytes
USE_F8 = int(os.environ.get("USE_F8", "1"))  # f8 gather tables (xe, h1)
F8_OH = int(os.environ.get("F8_OH", "1"))    # f8 one-hot (DVE output dtype)
F8_DR = int(os.environ.get("F8_DR", "0"))    # DoubleRow paired-chunk matmuls
                                             # (measured SLOWER on HW - keep off)
SOFTPIPE = int(os.environ.get("SOFTPIPE", "1"))  # cross-rep pipelining of the
                                             # timing-replica loop
GBUFS = int(os.environ.get("GBUFS", "3"))    # gather/one-hot pool depth
                                             # (3 pipelines the L3/L1 pool
                                             # handoff across reps; ~60us/rep
                                             # better than 2 measured at R=12)
ABUFS = int(os.environ.get("ABUFS", "2"))    # node-major rebuild pool depth
TBUFS = int(os.environ.get("TBUFS", "4"))    # aggT evacuation pool depth


# --------------------------------------------------------------------------
# Host-side preprocessing: shard edges, build gather-index / one-hot inputs
# --------------------------------------------------------------------------

def preprocess(edge_index, n_nodes):
    src = np.asarray(edge_index[0], dtype=np.int64)
    dst = np.asarray(edge_index[1], dtype=np.int64)
    S = n_nodes // NCORES
    NW = (S + P - 1) // P
    NB = (NW + BPB - 1) // BPB
    HALF = n_nodes // 2
    assert HALF < 32768 and n_nodes - HALF < 32768

    deg = np.bincount(dst, minlength=n_nodes).astype(np.float32)
    invdeg = (1.0 / np.maximum(deg, 1.0)).astype(np.float32)

    core = dst // S
    local = dst - core * S
    w = local // P
    slot = local % P
    blk = w // BPB
    wrel = w - blk * BPB
    half = (src >= HALF).astype(np.int64)
    idxval = (src - half * HALF).astype(np.int64)

    # group key (core, blk, half, wrel); stable sort groups the edges
    key = ((core * NB + blk) * 2 + half) * BPB + wrel
    order = np.argsort(key, kind="stable")
    skey = key[order]
    sidx = idxval[order]
    sslot = slot[order]
    ssrc = src[order]

    ngroups = NCORES * NB * 2 * BPB
    counts = np.bincount(skey, minlength=ngroups).reshape(NCORES, NB, 2, BPB)
    starts = np.zeros(ngroups + 1, dtype=np.int64)
    np.cumsum(counts.reshape(-1), out=starts[1:])

    # uniform chunk counts across cores (SPMD: one NEFF for all 8 cores).
    # min 1 chunk per (blk, half, window) group so every psum region is
    # written (all-pad chunks contribute zero).
    C = np.ceil(counts.max(axis=0) / P).astype(np.int64)  # [NB, 2, BPB]
    C[: (NW // BPB)] = np.maximum(C[: (NW // BPB)], 1)
    if NW % BPB:
        C[-1, :, : NW % BPB] = np.maximum(C[-1, :, : NW % BPB], 1)
    CTOT = int(C.sum())
    LTOT = CTOT * P

    # chunk offset of each (blk, half, wrel) group in the concatenated arrays
    chunk_off = np.zeros((NB, 2, BPB), dtype=np.int64)
    acc = 0
    for b in range(NB):
        for h in range(2):
            for r in range(BPB):
                chunk_off[b, h, r] = acc
                acc += C[b, h, r]

    idx_arrs, dslot_arrs, invd_arrs, invdT_arrs, gsrc_arrs = [], [], [], [], []
    for c in range(NCORES):
        idx_a = np.zeros((16, LTOT // 16), dtype=np.int16)
        ds_a = np.full((P, CTOT), -1.0, dtype=np.float16)
        gs_a = np.full((CTOT, P), -1, dtype=np.int64)
        for b in range(NB):
            for h in range(2):
                for r in range(BPB):
                    cg = int(C[b, h, r])
                    if cg == 0:
                        continue
                    g = ((c * NB + b) * 2 + h) * BPB + r
                    s0, s1 = starts[g], starts[g + 1]
                    k = s1 - s0
                    Lg = cg * P
                    co = int(chunk_off[b, h, r])
                    buf = np.zeros(Lg, dtype=np.int16)
                    buf[:k] = sidx[s0:s1].astype(np.int16)
                    idx_a[:, co * 8 : co * 8 + Lg // 16] = buf.reshape(-1, 16).T
                    sl = np.full(Lg, -1.0, dtype=np.float16)
                    sl[:k] = sslot[s0:s1].astype(np.float16)
                    ds_a[:, co : co + cg] = sl.reshape(cg, P).T
                    gs = np.full(Lg, -1, dtype=np.int64)
                    gs[:k] = ssrc[s0:s1]
                    gs_a[co : co + cg, :] = gs.reshape(cg, P)
        idx_arrs.append(np.tile(idx_a, (8, 1)))  # replicate to 128 partitions
        dslot_arrs.append(ds_a)
        gsrc_arrs.append(gs_a)

        # invdeg per core: [128, NW] (partition=slot, col=window), pad 1.0
        v = np.ones(NW * P, dtype=np.float32)
        v[:S] = invdeg[c * S : (c + 1) * S]
        invd_arrs.append(v.reshape(NW, P).T.copy())
        # feature-major inv-degree row, replicated to 128 partitions
        vt = np.ones(S, dtype=np.float16)
        vt[:] = invdeg[c * S : (c + 1) * S]
        invdT_arrs.append(np.broadcast_to(vt, (P, S)).copy())

    meta = dict(N=n_nodes, S=S, NW=NW, NB=NB, HALF=HALF, C=C,
                chunk_off=chunk_off, CTOT=CTOT, ITOT=LTOT // 16)
    return meta, dict(idx=idx_arrs, dslot=dslot_arrs, invd=invd_arrs,
                      invdT=invdT_arrs, gsrc=gsrc_arrs)


# --------------------------------------------------------------------------
# Device program
# --------------------------------------------------------------------------

def build_program(meta, INDIM, HID, OUT, reps=1, ncores=NCORES, mock_cc=False,
                  abl=()):
    N, S, NW, NB, HALF = (meta["N"], meta["S"], meta["NW"], meta["NB"],
                          meta["HALF"])
    C, chunk_off, CTOT, ITOT = (meta["C"], meta["chunk_off"], meta["CTOT"],
                                meta["ITOT"])
    LASTW = S - P * (NW - 1)
    OUTP = 64  # padded projection width for layer 3
    PTW = 128  # p-table row width (f16, 256B - dma_gather minimum)
    RG = [list(range(ncores))]

    nc = bacc.Bacc("TRN2", target_bir_lowering=False, debug=False,
                   num_devices=ncores, num_swdge_queues=4,
                   dynamic_dma_scratch_size=DSCRATCH)
    qctr = [0]
    GDT = F8 if USE_F8 else F16  # dtype of gathered neighbor rows (L1/L2)
    OHDT = F8 if (USE_F8 and F8_OH) else F16  # one-hot dtype
    use_dr = bool(USE_F8 and F8_OH and F8_DR)  # DoubleRow needs both f8
    DR = mybir.MatmulPerfMode.DoubleRow

    # ---- I/O ----
    xe_d = nc.dram_tensor("xe", [P, CTOT * INDIM], GDT, kind="ExternalInput")
    xT = nc.dram_tensor("xT", [P, S], F16, kind="ExternalInput")
    idx_d = nc.dram_tensor("idx", [P, ITOT], I16, kind="ExternalInput")
    dslot_d = nc.dram_tensor("dslot", [P, CTOT], F16, kind="ExternalInput")
    invd_d = nc.dram_tensor("invd", [P, NW], F32, kind="ExternalInput")
    invdT_d = nc.dram_tensor("invdT", [P, S], F16, kind="ExternalInput")
    w1l_d = nc.dram_tensor("w1l", [INDIM, HID], F16, kind="ExternalInput")
    w1r_d = nc.dram_tensor("w1r", [INDIM, HID], F16, kind="ExternalInput")
    w2l_d = nc.dram_tensor("w2l", [HID, HID], F16, kind="ExternalInput")
    w2r_d = nc.dram_tensor("w2r", [HID, HID], F16, kind="ExternalInput")
    w3l_d = nc.dram_tensor("w3l", [HID, OUTP], F16, kind="ExternalInput")
    w3r_d = nc.dram_tensor("w3r", [HID, OUT], F16, kind="ExternalInput")
    g1_d = nc.dram_tensor("g1", [P, 2], F32, kind="ExternalInput")
    be1_d = nc.dram_tensor("be1", [P, 2], F32, kind="ExternalInput")
    g2_d = nc.dram_tensor("g2", [P, 2], F32, kind="ExternalInput")
    be2_d = nc.dram_tensor("be2", [P, 2], F32, kind="ExternalInput")
    b3_d = nc.dram_tensor("b3", [P, 1], F32, kind="ExternalInput")
    out_d = nc.dram_tensor("out", [S, OUT], F32, kind="ExternalOutput")

    from contextlib import ExitStack

    with tile.TileContext(nc) as tc, ExitStack() as es:
        cp = es.enter_context(tc.tile_pool(name="const", bufs=1))
        gp = es.enter_context(tc.tile_pool(name="gath", bufs=GBUFS))
        ohp = es.enter_context(tc.tile_pool(name="oh", bufs=GBUFS))
        agp = es.enter_context(tc.tile_pool(name="agg", bufs=ABUFS))
        bigp = es.enter_context(tc.tile_pool(name="big", bufs=6))
        sqp = es.enter_context(tc.tile_pool(name="sq", bufs=2))
        smp = es.enter_context(tc.tile_pool(name="small", bufs=2))
        pseg = es.enter_context(tc.tile_pool(name="pseg", bufs=4, space="PSUM"))
        ptr = es.enter_context(tc.tile_pool(name="ptr", bufs=2, space="PSUM"))
        pz = es.enter_context(tc.tile_pool(name="pz", bufs=2, space="PSUM"))
        atp = es.enter_context(tc.tile_pool(name="aggT", bufs=TBUFS))
        drp = es.enter_context(tc.tile_pool(name="dram", bufs=1, space="DRAM"))

        # ---- constants ----
        idx_sb = cp.tile([P, ITOT], I16, name="idx_sb")
        nc.sync.dma_start(idx_sb[:], idx_d[:, :])

        invd_sb = cp.tile([P, NW], F32, name="invd_sb")
        nc.sync.dma_start(invd_sb[:], invd_d[:, :])
        invdT_sb = cp.tile([P, S], F16, name="invdT_sb")
        nc.sync.dma_start(invdT_sb[:], invdT_d[:, :])
        xT_sb = cp.tile([P, S], F16, name="xT_sb")
        nc.sync.dma_start(xT_sb[:], xT[:, :])

        w1l_sb = cp.tile([P, HID], F16, name="w1l_sb")
        nc.sync.dma_start(w1l_sb[:], w1l_d[:, :])
        w1r_sb = cp.tile([P, HID], F16, name="w1r_sb")
        nc.sync.dma_start(w1r_sb[:], w1r_d[:, :])
        w2l_sb = [cp.tile([P, HID], F16, name=f"w2l_sb{k}") for k in range(2)]
        w2r_sb = [cp.tile([P, HID], F16, name=f"w2r_sb{k}") for k in range(2)]
        w3l_sb = [cp.tile([P, OUTP], F16, name=f"w3l_sb{k}") for k in range(2)]
        w3r_sb = [cp.tile([P, OUT], F16, name=f"w3r_sb{k}") for k in range(2)]
        for k in range(2):
            nc.sync.dma_start(w2l_sb[k][:], w2l_d[k * P : (k + 1) * P, :])
            nc.sync.dma_start(w2r_sb[k][:], w2r_d[k * P : (k + 1) * P, :])
            nc.sync.dma_start(w3l_sb[k][:], w3l_d[k * P : (k + 1) * P, :])
            nc.sync.dma_start(w3r_sb[k][:], w3r_d[k * P : (k + 1) * P, :])
        g1_sb = cp.tile([P, 2], F32, name="g1_sb")
        nc.sync.dma_start(g1_sb[:], g1_d[:, :])
        be1_sb = cp.tile([P, 2], F32, name="be1_sb")
        nc.sync.dma_start(be1_sb[:], be1_d[:, :])
        g2_sb = cp.tile([P, 2], F32, name="g2_sb")
        nc.sync.dma_start(g2_sb[:], g2_d[:, :])
        be2_sb = cp.tile([P, 2], F32, name="be2_sb")
        nc.sync.dma_start(be2_sb[:], be2_d[:, :])
        b3_sb = cp.tile([P, 1], F32, name="b3_sb")
        nc.sync.dma_start(b3_sb[:], b3_d[:, :])

        iota_i = cp.tile([P, P], I32, name="iota_i")
        nc.gpsimd.iota(iota_i[:], pattern=[[1, P]], base=0, channel_multiplier=0)
        iota_h = cp.tile([P, P], F16, name="iota_h")
        nc.vector.tensor_copy(iota_h[:], iota_i[:])
        dslot_h = cp.tile([P, CTOT], F16, name="dslot_h")
        nc.sync.dma_start(dslot_h[:], dslot_d[:, :])
        ident = cp.tile([P, P], F32, name="ident")
        make_identity(nc, ident[:])
        ident_h = cp.tile([P, P], F16, name="ident_h")
        nc.vector.tensor_copy(ident_h[:], ident[:])
        eps_sb = cp.tile([P, 1], F32, name="eps_sb")
        nc.vector.memset(eps_sb[:], EPS)

        def wins(b):
            return list(range(b * BPB, min(b * BPB + BPB, NW)))

        def load_chunks_l1(b, h, lname):
            """Sequential DMA of host-pregathered x rows for (block, half).
            Returns (g_view [P, nch, INDIM], chunk offset)."""
            co = int(chunk_off[b, h, 0])
            nch = int(C[b, h].sum())
            g_t = gp.tile([P, nch * INDIM], GDT, tag="g", name=f"g{lname}_{b}_{h}")
            if "nogather" in abl:
                nc.vector.memset(g_t[:, 0:1], 0.0)
            else:
                nc.sync.dma_start(
                    g_t[:], xe_d[:, co * INDIM : (co + nch) * INDIM])
            return g_t[:].rearrange("p (c f) -> p c f", f=INDIM), co

        def gather_chunks(b, h, F, base_ap, lname, dt=F16):
            """dma_gather of (block, half) chunk rows from a DRAM table.
            Returns (g_view [P, nch, F], chunk offset)."""
            co = int(chunk_off[b, h, 0])
            nch = int(C[b, h].sum())
            g_t = gp.tile([P, nch * F], dt, tag="g", name=f"g{lname}_{b}_{h}")
            gv = g_t[:].rearrange("p (c f) -> p c f", f=F)
            if "nogather" in abl:
                nc.vector.memset(g_t[:, 0:1], 0.0)
                return gv, co
            npieces = -(-nch // MAXC)
            sizes = [nch // npieces + (1 if i < nch % npieces else 0)
                     for i in range(npieces)]
            offs = [sum(sizes[:i]) for i in range(npieces)]
            for c0, cn in zip(offs, sizes):
                nc.gpsimd.dma_gather(
                    out_ap=gv[:, c0 : c0 + cn, :],
                    in_ap=base_ap,
                    idxs_ap=idx_sb[:, (co + c0) * 8 : (co + c0 + cn) * 8],
                    num_idxs=cn * P,
                    num_idxs_reg=cn * P,
                    elem_size=F,
                    queue_num=qctr[0] % 4,
                )
                qctr[0] += 1
            return gv, co

        def build_onehot(b, h, lname, dt=F16):
            """One-hot (edge -> dst slot) for all chunks of (block, half).
            Built on DVE (is_equal is DVE-only)."""
            co = int(chunk_off[b, h, 0])
            nch = int(C[b, h].sum())
            oh_t = ohp.tile([P, nch * P], dt, tag="oh", name=f"oh{lname}_{b}_{h}")
            ohv = oh_t[:].rearrange("p (c q) -> p c q", q=P)
            if "noonehot" in abl:
                nc.vector.memset(oh_t[:, 0:1], 0.0)
            else:
                nc.vector.tensor_tensor(
                    out=ohv,
                    in0=iota_h[:].unsqueeze(1).to_broadcast([P, nch, P]),
                    in1=dslot_h[:, co : co + nch].unsqueeze(2).to_broadcast(
                        [P, nch, P]),
                    op=AL.is_equal,
                )
            return ohv

        def segsum_block(b, F, lname, base_lo=None, base_hi=None):
            """Feature-major segment-sum psums for one block of windows.
            Returns {(h, hh): psum [P, BPB*P]} (partition=feature, cols =
            windows-of-block x dst slot), one per (src-half, feature-half),
            unscaled.  Each psum accumulation group opens and closes within
            one window's chunk run of one half (PSUM allows only one open
            group per bank region)."""
            nh = F // P
            wl = wins(b)
            pss = [pseg.tile([P, BPB * P], F32, tag="seg",
                             name=f"ps{lname}_{b}_{hh}") for hh in range(nh)]
            gvs = {}
            ohvs = {}
            for h, base in ((0, base_lo), (1, base_hi)):
                if int(C[b, h].sum()) == 0:
                    continue
                if lname == "L1":
                    gvs[h], _ = load_chunks_l1(b, h, lname)
                else:
                    gvs[h], _ = gather_chunks(b, h, F, base, lname, dt=GDT)
                ohvs[h] = build_onehot(b, h, lname, dt=OHDT)
            # one window's psum group opens and closes fully (both halves)
            # before the next window's group starts in the same bank
            for r in range(len(wl)):
                halves = [h for h in (0, 1) if int(C[b, h, r]) > 0 and h in gvs]
                for h in halves:
                    co_r = int(chunk_off[b, h, r] - chunk_off[b, h, 0])
                    cg = int(C[b, h, r])
                    j = 0
                    while j < cg:
                        if "nosegmm" in abl:
                            break
                        # f8: fold chunk pairs into one DoubleRow matmul
                        # (PE sums both 128-deep k-tiles in a single pass)
                        pair = 2 if (use_dr and j + 1 < cg) else 1
                        st = h == halves[0] and j == 0
                        sp = h == halves[-1] and j + pair == cg
                        for hh in range(nh):
                            if pair == 2:
                                nc.tensor.matmul(
                                    out=pss[hh][:, r * P : (r + 1) * P],
                                    lhsT=gvs[h][:, co_r + j : co_r + j + 2,
                                                hh * P : (hh + 1) * P],
                                    rhs=ohvs[h][:, co_r + j : co_r + j + 2, :],
                                    start=st, stop=sp, perf_mode=DR,
                                )
                            else:
                                nc.tensor.matmul(
                                    out=pss[hh][:, r * P : (r + 1) * P],
                                    lhsT=gvs[h][:, co_r + j,
                                                hh * P : (hh + 1) * P],
                                    rhs=ohvs[h][:, co_r + j, :],
                                    start=st, stop=sp,
                                )
                        j += pair
            return pss

        def dense_layer(lname, F_in, hT, wl_sb, wr_sb, base_lo=None,
                        base_hi=None):
            """Full SAGE layer (aggregate + dense), feature-major output."""
            nh = F_in // P
            zT = [bigp.tile([P, S], F16, tag="big", name=f"zT{lname}_{m}")
                  for m in range(2)]
            szp = [smp.tile([P, NB], F32, tag=f"szp{lname}{m}",
                            name=f"szp{lname}{m}") for m in range(2)]
            ssqp = [smp.tile([P, NB], F32, tag=f"ssqp{lname}{m}",
                             name=f"ssqp{lname}{m}") for m in range(2)]
            for b in range(NB):
                n0 = b * BLK
                nb = min(S, n0 + BLK) - n0
                wl = wins(b)
                pss = segsum_block(b, F_in, lname, base_lo, base_hi)
                aggT = [atp.tile([P, BLK], F16, tag="aggT",
                                 name=f"aggT{lname}_{b}_{hh}")
                        for hh in range(nh)]
                for hh in range(nh):
                    nc.vector.tensor_tensor(
                        out=aggT[hh][:, :nb],
                        in0=pss[hh][:, :nb],
                        in1=invdT_sb[:, n0 : n0 + nb],
                        op=AL.mult,
                    )
                for m in range(2):
                    psz = pz.tile([P, BLK], F32, tag="z",
                                  name=f"pz{lname}_{b}_{m}")
                    mcols = slice(m * P, (m + 1) * P)
                    nmm = 2 * nh
                    k = 0
                    for hh in range(nh):
                        nc.tensor.matmul(
                            out=psz[:, :nb],
                            lhsT=wl_sb[hh][:, mcols],
                            rhs=aggT[hh][:, :nb],
                            start=(k == 0), stop=(k == nmm - 1),
                        )
                        k += 1
                    for hh in range(nh):
                        nc.tensor.matmul(
                            out=psz[:, :nb],
                            lhsT=wr_sb[hh][:, mcols],
                            rhs=hT[hh][:, n0 : n0 + nb],
                            start=(k == 0), stop=(k == nmm - 1),
                        )
                        k += 1
                    # fused: zT copy + per-block sum on ACT (frees DVE)
                    nc.scalar.activation(
                        out=zT[m][:, n0 : n0 + nb], in_=psz[:, :nb],
                        func=AF.Copy, accum_out=szp[m][:, b : b + 1],
                    )
                    sqsc = sqp.tile([P, BLK], F16, tag="sq",
                                    name=f"sq{lname}_{b}_{m}")
                    nc.scalar.activation(
                        out=sqsc[:, :nb], in_=psz[:, :nb], func=AF.Square,
                        accum_out=ssqp[m][:, b : b + 1],
                    )
            return zT, szp, ssqp

        def collective(kind, op, ins, outs, tag=None):
            if mock_cc or (tag is not None and f"mock_{tag}" in abl):
                n_in = ins[0].shape[0]
                n_out = outs[0].shape[0]
                for k in range(max(1, n_out // n_in)):
                    nc.sync.dma_start(outs[0][k * n_in : (k + 1) * n_in],
                                      ins[0])
            else:
                nc.gpsimd.collective_compute(kind, op, replica_groups=RG,
                                             ins=ins, outs=outs)

        def bn_stats(li, lname, szp, ssqp, st_in, st_out):
            """Reduce per-block partial sums and issue the stats AllReduce."""
            stat = smp.tile([P, 4], F32, tag=f"stat{lname}", name=f"stat{lname}")
            for m in range(2):
                nc.vector.tensor_reduce(out=stat[:, m : m + 1], in_=szp[m][:],
                                        axis=mybir.AxisListType.X, op=AL.add)
                nc.vector.tensor_reduce(out=stat[:, 2 + m : 3 + m],
                                        in_=ssqp[m][:],
                                        axis=mybir.AxisListType.X, op=AL.add)
            nc.sync.dma_start(st_in[li][:], stat[:])
            collective("AllReduce", AL.add, [st_in[li][:]], [st_out[li][:]],
                       tag="ar")

        def bn_apply(li, lname, zT, g_sb, be_sb, st_out):
            """hT = relu((z - mean) * a + be) from the AllReduced stats."""
            statg = smp.tile([P, 4], F32, tag=f"statg{lname}", name=f"statg{lname}")
            nc.sync.dma_start(statg[:], st_out[li][:])
            mean, a_m = [], []
            for m in range(2):
                mn = smp.tile([P, 1], F32, tag=f"mean{lname}{m}",
                              name=f"mean{lname}{m}")
                nc.vector.tensor_scalar(out=mn[:], in0=statg[:, m : m + 1],
                                        scalar1=1.0 / N, scalar2=None,
                                        op0=AL.mult)
                ex2 = smp.tile([P, 1], F32, tag=f"ex2{lname}{m}",
                               name=f"ex2{lname}{m}")
                nc.vector.tensor_scalar(out=ex2[:], in0=statg[:, 2 + m : 3 + m],
                                        scalar1=1.0 / N, scalar2=None,
                                        op0=AL.mult)
                nvar = smp.tile([P, 1], F32, tag=f"nvar{lname}{m}",
                                name=f"nvar{lname}{m}")
                # nvar = mean^2 - E[x^2]  (= -var)
                nc.vector.scalar_tensor_tensor(
                    out=nvar[:], in0=mn[:], scalar=mn[:], in1=ex2[:],
                    op0=AL.mult, op1=AL.subtract,
                )
                # istd = exp(-0.5 * ln(var + eps)) - stays in the ln/exp
                # activation table set (no table swap for sqrt)
                lnv = smp.tile([P, 1], F32, tag=f"lnv{lname}{m}",
                               name=f"lnv{lname}{m}")
                nc.scalar.activation(out=lnv[:], in_=nvar[:], func=AF.Ln,
                                     bias=eps_sb[:], scale=-1.0)
                istd = smp.tile([P, 1], F32, tag=f"istd{lname}{m}",
                                name=f"istd{lname}{m}")
                nc.scalar.activation(out=istd[:], in_=lnv[:], func=AF.Exp,
                                     scale=-0.5)
                am = smp.tile([P, 1], F32, tag=f"a{lname}{m}",
                              name=f"a{lname}{m}")
                nc.vector.tensor_tensor(out=am[:], in0=g_sb[:, m : m + 1],
                                        in1=istd[:], op=AL.mult)
                mean.append(mn)
                a_m.append(am)
            hT = []
            for m in range(2):
                h_m = bigp.tile([P, S], F16, tag="big", name=f"hT{lname}_{m}")
                nc.vector.tensor_scalar(out=h_m[:], in0=zT[m][:],
                                        scalar1=mean[m][:], scalar2=a_m[m][:],
                                        op0=AL.subtract, op1=AL.mult)
                hT.append(h_m)
            for m in range(2):
                nc.scalar.activation(out=hT[m][:], in_=hT[m][:], func=AF.Relu,
                                     bias=be_sb[:, m : m + 1], scale=1.0)
            return hT

        def emit_front_a(rep):
            # ---- DRAM intermediates (fresh per rep: Shared tiles are
            # single-writer) ----
            shr = {} if mock_cc else dict(addr_space="Shared")
            st = dict(rep=rep)
            st["h1_shard"] = drp.tile([S, HID], GDT, name=f"h1_shard_{rep}")
            st["h1_full"] = drp.tile([N, HID], GDT, name=f"h1_full_{rep}",
                                     **shr)
            st["p_shard"] = drp.tile([S, PTW], F16, name=f"p_shard_{rep}")
            st["p_full"] = drp.tile([N, PTW], F16, name=f"p_full_{rep}", **shr)
            st["st_in"] = [drp.tile([P, 4], F32, name=f"st_in{l}_{rep}")
                           for l in range(2)]
            st["st_out"] = [drp.tile([P, 4], F32, name=f"st_out{l}_{rep}")
                            for l in range(2)]
            h1_shard, h1_full = st["h1_shard"], st["h1_full"]
            st_in, st_out = st["st_in"], st["st_out"]

            # ================= Layer 1 =================
            zT1, szp1, ssqp1 = dense_layer("L1", INDIM, [xT_sb], [w1l_sb],
                                           [w1r_sb])
            bn_stats(0, "L1", szp1, ssqp1, st_in, st_out)
            st["zT1"] = zT1
            return st

        def emit_front_b(st):
            h1_shard, h1_full = st["h1_shard"], st["h1_full"]
            st_in, st_out = st["st_in"], st["st_out"]
            h1T = bn_apply(0, "L1", st["zT1"], g1_sb, be1_sb, st_out)

            # rebuild node-major h1 and AllGather the full table
            for wi in range(NW):
                rows = P if wi < NW - 1 else LASTW
                hnm = agp.tile([P, HID], GDT, tag="agg", name=f"hnm_{wi}")
                for hh in range(2):
                    pst = ptr.tile([P, P], F16, tag="tr", name=f"ptrh_{wi}_{hh}")
                    nc.tensor.transpose(pst[:rows, :],
                                        h1T[hh][:, wi * P : wi * P + rows],
                                        ident_h[:])
                    if hh == 0:
                        nc.scalar.activation(
                            out=hnm[:rows, hh * P : (hh + 1) * P],
                            in_=pst[:rows, :], func=AF.Copy)
                    else:
                        nc.vector.tensor_copy(
                            out=hnm[:rows, hh * P : (hh + 1) * P],
                            in_=pst[:rows, :])
                nc.sync.dma_start(h1_shard[wi * P : wi * P + rows, :],
                                  hnm[:rows, :])
            collective("AllGather", AL.bypass, [h1_shard[:, :]], [h1_full[:, :]],
                       tag="ag1")
            st["h1T"] = h1T
            return st

        def emit_mid_a(st):
            h1_full = st["h1_full"]
            st_in, st_out = st["st_in"], st["st_out"]
            h1T = st["h1T"]
            # ================= Layer 2 =================
            zT2, szp2, ssqp2 = dense_layer("L2", HID, h1T, w2l_sb, w2r_sb,
                                           base_lo=h1_full[:, :],
                                           base_hi=h1_full[HALF:N, :])
            bn_stats(1, "L2", szp2, ssqp2, st_in, st_out)
            st["zT2"] = zT2
            return st

        def emit_mid_b(st):
            p_shard = st["p_shard"]
            st_out = st["st_out"]
            h2T = bn_apply(1, "L2", st["zT2"], g2_sb, be2_sb, st_out)

            # ================= Layer 3 =================
            # p = h2 @ w3l (padded to 64 cols), rebuilt node-major + AllGather
            pT = bigp.tile([P, S], F16, tag="big", name="pT")  # rows :64 used
            for b in range(NB):
                n0 = b * BLK
                nb = min(S, n0 + BLK) - n0
                psp = pz.tile([P, BLK], F32, tag="z", name=f"pzp_{b}")
                for hh in range(2):
                    nc.tensor.matmul(
                        out=psp[:OUTP, :nb], lhsT=w3l_sb[hh][:],
                        rhs=h2T[hh][:, n0 : n0 + nb],
                        start=(hh == 0), stop=(hh == 1),
                    )
                nc.vector.tensor_copy(out=pT[:OUTP, n0 : n0 + nb],
                                      in_=psp[:OUTP, :nb])
            for wi in range(NW):
                rows = P if wi < NW - 1 else LASTW
                pnm = agp.tile([P, PTW], F16, tag="agg", name=f"pnm_{wi}")
                nc.vector.memset(pnm[:, OUTP:], 0.0)
                pst = ptr.tile([P, P], F16, tag="tr", name=f"ptrp_{wi}")
                nc.tensor.transpose(pst[:rows, :OUTP],
                                    pT[:OUTP, wi * P : wi * P + rows],
                                    ident_h[:OUTP, :OUTP])
                if wi % 2:
                    nc.scalar.activation(out=pnm[:rows, :OUTP],
                                         in_=pst[:rows, :OUTP], func=AF.Copy)
                else:
                    nc.vector.tensor_copy(out=pnm[:rows, :OUTP],
                                          in_=pst[:rows, :OUTP])
                nc.sync.dma_start(p_shard[wi * P : wi * P + rows, :],
                                  pnm[:rows, :])
            collective("AllGather", AL.bypass, [p_shard[:, :]],
                       [st["p_full"][:, :]], tag="ag2")

            # z3r^T = w3r^T @ h2^T + b3 (feature-major, 47 rows)
            z3rT = bigp.tile([P, S], F16, tag="big", name="z3rT")
            for b in range(NB):
                n0 = b * BLK
                nb = min(S, n0 + BLK) - n0
                psr = pz.tile([P, BLK], F32, tag="z", name=f"pzr_{b}")
                for hh in range(2):
                    nc.tensor.matmul(
                        out=psr[:OUT, :nb], lhsT=w3r_sb[hh][:],
                        rhs=h2T[hh][:, n0 : n0 + nb],
                        start=(hh == 0), stop=(hh == 1),
                    )
                nc.vector.tensor_scalar(out=z3rT[:OUT, n0 : n0 + nb],
                                        in0=psr[:OUT, :nb],
                                        scalar1=b3_sb[:OUT, :], scalar2=None,
                                        op0=AL.add)

            st["z3rT"] = z3rT
            return st

        def emit_back(st):
            p_full, z3rT = st["p_full"], st["z3rT"]
            # aggregate p (node-major psums), softmax pass A per window
            out_sb = bigp.tile([P, NW * OUT], F32, tag="outsb", bufs=1,
                               name="out_sb")
            negmax_all = smp.tile([P, NW], F32, tag="negmax", name="negmax_all")
            sume_all = smp.tile([P, NW], F32, tag="sume", name="sume_all")
            nc.vector.memset(sume_all[:], 1.0)  # pad rows: ln(1)=0
            p_lo = p_full[:, :]
            p_hi = p_full[HALF:N, :]
            for b in range(NB):
                wl = wins(b)
                ps3 = pseg.tile([P, BPB * OUTP], F32, tag="seg",
                                name=f"ps3_{b}")
                gvs = {}
                ohvs = {}
                for h, base in ((0, p_lo), (1, p_hi)):
                    if int(C[b, h].sum()) == 0:
                        continue
                    gvs[h], _ = gather_chunks(b, h, PTW, base, "L3")
                    ohvs[h] = build_onehot(b, h, "L3")
                for r in range(len(wl)):
                    halves = [h for h in (0, 1)
                              if int(C[b, h, r]) > 0 and h in gvs]
                    for h in halves:
                        co_r = int(chunk_off[b, h, r] - chunk_off[b, h, 0])
                        cg = int(C[b, h, r])
                        for j in range(cg):
                            if "nosegmm" in abl:
                                continue
                            nc.tensor.matmul(
                                out=ps3[:, r * OUTP : (r + 1) * OUTP],
                                lhsT=ohvs[h][:, co_r + j, :],
                                rhs=gvs[h][:, co_r + j, :OUTP],
                                start=(h == halves[0] and j == 0),
                                stop=(h == halves[-1] and j == cg - 1),
                            )
                for r in range(len(wl)):
                    wi = b * BPB + r
                    rows = P if wi < NW - 1 else LASTW
                    pst = ptr.tile([P, P], F16, tag="tr", name=f"ptrz_{wi}")
                    nc.tensor.transpose(pst[:rows, :OUT],
                                        z3rT[:OUT, wi * P : wi * P + rows],
                                        ident_h[:OUT, :OUT])
                    z3nm = agp.tile([P, OUT], F16, tag="agg", name=f"z3nm_{wi}")
                    nc.scalar.activation(out=z3nm[:rows, :],
                                         in_=pst[:rows, :OUT], func=AF.Copy)
                    osl = out_sb[:rows, wi * OUT : (wi + 1) * OUT]
                    # z3w = agg * invd + z3r
                    nc.vector.scalar_tensor_tensor(
                        out=osl, in0=ps3[:rows, r * OUTP : r * OUTP + OUT],
                        scalar=invd_sb[:rows, wi : wi + 1],
                        in1=z3nm[:rows, :], op0=AL.mult, op1=AL.add,
                    )
                    nc.vector.tensor_reduce(
                        out=negmax_all[:rows, wi : wi + 1], in_=osl,
                        axis=mybir.AxisListType.X, op=AL.max, negate=True)
                    esc = smp.tile([P, OUT], F32, tag="esc", name=f"esc_{wi}")
                    nc.scalar.activation(
                        out=esc[:rows, :], in_=osl, func=AF.Exp,
                        bias=negmax_all[:rows, wi : wi + 1], scale=1.0,
                        accum_out=sume_all[:rows, wi : wi + 1])
            # single Ln pass, then combine
            logsum_all = smp.tile([P, NW], F32, tag="logsum", name="logsum_all")
            nc.scalar.activation(out=logsum_all[:], in_=sume_all[:], func=AF.Ln)
            for wi in range(NW):
                rows = P if wi < NW - 1 else LASTW
                nc.vector.tensor_scalar(
                    out=out_sb[:rows, wi * OUT : (wi + 1) * OUT],
                    in0=out_sb[:rows, wi * OUT : (wi + 1) * OUT],
                    scalar1=negmax_all[:rows, wi : wi + 1],
                    scalar2=logsum_all[:rows, wi : wi + 1],
                    op0=AL.add, op1=AL.subtract,
                )
            # store (full windows in one strided DMA, tail window separately)
            nfull = NW - 1
            nc.sync.dma_start(
                out_d[0 : nfull * P, :].rearrange("(w p) f -> p w f", p=P),
                out_sb[:].rearrange("p (w f) -> p w f", f=OUT)[:, :nfull, :],
            )
            nc.sync.dma_start(
                out_d[nfull * P : S, :],
                out_sb[:LASTW, nfull * OUT : NW * OUT],
            )

        if SOFTPIPE == 2 and reps > 1:
            # 4-way split: collective-queue order AR2(k), AR1(k+1), AG2(k),
            # AG1(k+1) - the tiny stats AllReduces never queue behind a big
            # AllGather.
            nxt = emit_front_a(0)
            emit_front_b(nxt)
            for k in range(reps):
                cur = nxt
                emit_mid_a(cur)
                nxt = emit_front_a(k + 1) if k + 1 < reps else None
                emit_mid_b(cur)
                if nxt is not None:
                    emit_front_b(nxt)
                emit_back(cur)
        elif SOFTPIPE and reps > 1:
            # 2-phase pipeline: rep k+1's whole layer-1 (incl. its stats AR
            # and h1 AllGather issue) sits between rep k's p-AllGather and
            # rep k's L3 consumers.
            nxt = emit_front_a(0)
            emit_front_b(nxt)
            for k in range(reps):
                cur = nxt
                emit_mid_a(cur)
                emit_mid_b(cur)
                if k + 1 < reps:
                    nxt = emit_front_a(k + 1)
                    emit_front_b(nxt)
                else:
                    nxt = None
                emit_back(cur)
        else:
            for k in range(reps):
                st = emit_front_a(k)
                emit_front_b(st)
                emit_mid_a(st)
                emit_mid_b(st)
                emit_back(st)

    nc.compile()
    return nc


# --------------------------------------------------------------------------
# Entry point
# --------------------------------------------------------------------------

def _make_in_maps(inputs, meta, arrs):
    N = meta["N"]
    S = meta["S"]
    x16 = np.asarray(inputs["x"], dtype=np.float16)
    OUT = np.asarray(inputs["b3"]).shape[0]
    HID = np.asarray(inputs["b1"]).shape[0]
    INDIM = x16.shape[1]
    CTOT = meta["CTOT"]

    def bn_pack(v):
        return np.ascontiguousarray(
            np.asarray(v, dtype=np.float32).reshape(2, P).T
        )

    w3l_pad = np.zeros((HID, 64), dtype=np.float16)
    w3l_pad[:, :OUT] = np.asarray(inputs["w3l"], dtype=np.float16)
    b3_pad = np.zeros((P, 1), dtype=np.float32)
    b3_pad[:OUT, 0] = np.asarray(inputs["b3"], dtype=np.float32)

    shared = dict(
        w1l=np.asarray(inputs["w1l"], np.float16),
        w1r=np.asarray(inputs["w1r"], np.float16),
        w2l=np.asarray(inputs["w2l"], np.float16),
        w2r=np.asarray(inputs["w2r"], np.float16),
        w3l=w3l_pad,
        w3r=np.asarray(inputs["w3r"], np.float16),
        g1=bn_pack(inputs["g1"]), be1=bn_pack(inputs["be1"]),
        g2=bn_pack(inputs["g2"]), be2=bn_pack(inputs["be2"]),
        b3=b3_pad,
    )
    # x rows padded with a zero row at index N for -1 (pad) gsrc entries
    gdt = mybir.dt.np(F8) if USE_F8 else np.float16
    xpad = np.concatenate([x16, np.zeros((1, INDIM), np.float16)],
                          axis=0).astype(gdt)
    in_maps = []
    for c in range(NCORES):
        m = dict(shared)
        gs = arrs["gsrc"][c]  # [CTOT, P] int64, -1 = pad
        xe = xpad[gs.reshape(-1)]                  # [CTOT*P, INDIM]
        xe = xe.reshape(CTOT, P, INDIM).transpose(1, 0, 2)  # [P, CTOT, INDIM]
        m["xe"] = np.ascontiguousarray(xe.reshape(P, CTOT * INDIM))
        m["xT"] = np.ascontiguousarray(x16[c * S : (c + 1) * S, :].T)
        m["idx"] = arrs["idx"][c]
        m["dslot"] = arrs["dslot"][c]
        m["invd"] = arrs["invd"][c]
        m["invdT"] = arrs["invdT"][c]
        in_maps.append(m)
    return in_maps


_CACHE = {}


def _get_compiled(inputs):
    N, INDIM = np.asarray(inputs["x"]).shape
    HID = np.asarray(inputs["b1"]).shape[0]
    OUT = np.asarray(inputs["b3"]).shape[0]
    ei = np.ascontiguousarray(np.asarray(inputs["edge_index"], dtype=np.int64))
    key = (N, INDIM, HID, OUT, hash(ei.tobytes()))
    meta, arrs = preprocess(ei, N)
    if key not in _CACHE:
        _CACHE[key] = build_program(meta, INDIM, HID, OUT)
    return _CACHE[key], meta, arrs


def kernel(**inputs):
    nc, meta, arrs = _get_compiled(inputs)
    in_maps = _make_in_maps(inputs, meta, arrs)
    res = run_bass_kernel_spmd(nc, in_maps, core_ids=list(range(NCORES)))
    return np.concatenate([r["out"] for r in res.results], axis=0)

